# revision 1
# baseline (speedup 1.0000x reference)
import numpy as np
import jax
import jax.numpy as jnp
from functools import partial

# AFNO2D on 8 NeuronCores.
# Shapes (hardcoded from the problem spec):
#   x:  (2, 256, 256, 512) f32
#   w1: (2, 8, 64, 64), b1: (2, 8, 64), w2: (2, 8, 64, 64), b2: (2, 8, 64)
# Reference keeps only the low 32x32 corner of rfft2(x) (ortho), runs a
# per-block complex 64->64->64 MLP on it, zero-pads, irfft2, adds residual.
# Because only 32x32 modes survive, the FFTs collapse to small DFT matmuls:
#   X = Fh @ x @ Fw^T          (Fh, Fw: 32x256 complex corner-DFT, 1/16 ortho)
#   y = Re(Bh @ S) @ Cr^T + Im(Bh @ S) @ Ci^T   (corner inverse, Hermitian-w)
# The pipeline is fully independent per 64-channel block -> shard the 8
# blocks across the 8 cores with zero collectives.

H = 256
W = 256
C = 512
KM = 32   # kept modes per axis
NB = 8    # num_blocks
BS = 64   # block size

def _dft_mats():
    k = np.arange(KM)[:, None].astype(np.float64)
    h = np.arange(H)[None, :].astype(np.float64)
    ang = 2.0 * np.pi * k * h / H
    s = 1.0 / np.sqrt(H)
    Fr = (np.cos(ang) * s).astype(np.float32)          # [32,256]
    Fi = (-np.sin(ang) * s).astype(np.float32)
    # inverse over h (full ifft, only 32 modes nonzero)
    Br = (np.cos(ang).T * s).astype(np.float32)        # [256,32]
    Bi = (np.sin(ang).T * s).astype(np.float32)
    # inverse over w (irfft with Hermitian extension; k=0 col: weight 1, Im dropped)
    wgt = np.where(np.arange(KM) == 0, 1.0, 2.0)[None, :]
    Cr = (np.cos(ang).T * s * wgt).astype(np.float32)  # [256,32]
    Ci = (-np.sin(ang).T * s * wgt).astype(np.float32)
    return Fr, Fi, Br, Bi, Cr, Ci

_FR, _FI, _BR, _BI, _CR, _CI = _dft_mats()


def _per_core(x, w1, b1, w2, b2):
    # x: [2, 256, 256, 64]; w1: [2, 64, 64]; b1: [2, 64]; w2: [2,64,64]; b2: [2,64]
    Fr = jnp.asarray(_FR); Fi = jnp.asarray(_FI)
    Br = jnp.asarray(_BR); Bi = jnp.asarray(_BI)
    Cr = jnp.asarray(_CR); Ci = jnp.asarray(_CI)
    # forward corner DFT over h: U[b,k,w,c]
    Ur = jnp.einsum('kh,bhwc->bkwc', Fr, x)
    Ui = jnp.einsum('kh,bhwc->bkwc', Fi, x)
    # over w: X[b,k,l,c]
    Xr = jnp.einsum('lw,bkwc->bklc', Fr, Ur) - jnp.einsum('lw,bkwc->bklc', Fi, Ui)
    Xi = jnp.einsum('lw,bkwc->bklc', Fr, Ui) + jnp.einsum('lw,bkwc->bklc', Fi, Ur)
    # complex block MLP (this core's block)
    act = lambda t: jax.nn.gelu(t, approximate=False)
    o1r = act(Xr @ w1[0] - Xi @ w1[1] + b1[0])
    o1i = act(Xi @ w1[0] + Xr @ w1[1] + b1[1])
    o2r = o1r @ w2[0] - o1i @ w2[1] + b2[0]
    o2i = o1i @ w2[0] + o1r @ w2[1] + b2[1]
    # inverse over h: T[b,h,l,c]
    Tr = jnp.einsum('hk,bklc->bhlc', Br, o2r) - jnp.einsum('hk,bklc->bhlc', Bi, o2i)
    Ti = jnp.einsum('hk,bklc->bhlc', Br, o2i) + jnp.einsum('hk,bklc->bhlc', Bi, o2r)
    # inverse over w (real output) + residual
    y = jnp.einsum('wl,bhlc->bhwc', Cr, Tr) + jnp.einsum('wl,bhlc->bhwc', Ci, Ti)
    return y + x


_pmapped = jax.pmap(_per_core)


def kernel(x, w1, b1, w2, b2):
    x = np.asarray(x, dtype=np.float32)
    # shard: block n -> core n
    xs = np.ascontiguousarray(
        x.reshape(2, H, W, NB, BS).transpose(3, 0, 1, 2, 4))   # [8,2,256,256,64]
    w1s = np.ascontiguousarray(np.asarray(w1).transpose(1, 0, 2, 3))
    b1s = np.ascontiguousarray(np.asarray(b1).transpose(1, 0, 2))
    w2s = np.ascontiguousarray(np.asarray(w2).transpose(1, 0, 2, 3))
    b2s = np.ascontiguousarray(np.asarray(b2).transpose(1, 0, 2))
    ys = _pmapped(xs, w1s, b1s, w2s, b2s)          # [8,2,256,256,64]
    ys = np.asarray(ys)
    y = ys.transpose(1, 2, 3, 0, 4).reshape(2, H, W, C)
    return np.ascontiguousarray(y.astype(np.float32))

